# revision 1
# baseline (speedup 1.0000x reference)
"""Trainium2 Bass kernel for nn_EnhancedExternalMemoryBank (retrieval_knn).

Sharding: one head per NeuronCore (8 heads, 8 cores). Queries are sliced
per-head on the host (layout only); all arithmetic (chunk-mean, score GEMM,
top-k, chunk gather) runs on-device. Host concatenates per-head outputs.

Per-core pipeline:
  phase 1: chunk-sum of mem_keys over c (GpSimd pairwise adds; ranking is
     scale-invariant so sum == mean) -> PE transpose -> ckT [64, S] resident
  phase 2: per 128-query tile:
     PE: scores = qT.T @ ckT (exact fp32) into PSUM; ACT drains PSUM->SBUF
     DVE: max8 per S/4-wide piece -> merge to exact top-8 row values;
          find_index8 per piece; min-merge of (index + piece offset) resolves
          cross-piece duplicates in jax's tie order (not-found = 65535 loses)
     GPSIMD: indirect DMA gathers the top-4 chunks (2KB each) of keys/vals
             straight from DRAM by index, one chunk column per instruction
     contiguous 1MB DMA writes the gathered rows to the DRAM outputs

The kernel is DVE-bound: the two exact full scans (max8 + find_index8) over
the 4096x16384 fp32 score matrix are the floor (~1.1 ms/core); GEMM, drains,
sums, and all DMA overlap underneath it.
"""

import sys

sys.path.insert(0, "/opt/trn_rl_repo")

from contextlib import ExitStack

import numpy as np

import concourse.bass as bass
import concourse.tile as tile
from concourse import bacc, mybir
from concourse.bass import IndirectOffsetOnAxis
from concourse.bass_utils import run_bass_kernel_spmd
from concourse.masks import make_identity

F32 = mybir.dt.float32
F32R = mybir.dt.float32r
U16 = mybir.dt.uint16
U32 = mybir.dt.uint32

# Problem geometry (hardcoded per spec)
H, S, C, Dh = 8, 16384, 8, 64
L, B, Dm = 2048, 2, 512
N = L * B  # 4096 queries
KPC = 4  # chunks retrieved per query (k_per_chunk)
K = KPC * C  # 32
CHUNK = C * Dh  # 512 f32 = 2KB per chunk

# 'f32'  : exact fp32 matmul (4 cycles/row on PE)
# 'f32r' : single-pass fp32r matmul (1 cycle/row at N>=256; reduced precision,
#          needs producers rounded to fp32r — walrus verifier enforces)
MM_DTYPE = "f32"  # 'f32r' measured broken on HW: garbage output on every row


def retrieval_kernel(tc, qT, keys, vals, outk, outv, S_, N_, repeat=1):
    """Emit the per-core program.

    qT   : DRAM [Dh, N_] f32      (queries for this head, transposed)
    keys : DRAM [S_, CHUNK] f32   (mem_keys[h] flattened per chunk)
    vals : DRAM [S_, CHUNK] f32
    outk : DRAM [N_, KPC*CHUNK] f32
    outv : DRAM [N_, KPC*CHUNK] f32
    """
    for _rep in range(repeat):
        _emit_once(tc, qT, keys, vals, outk, outv, S_, N_)


def _emit_once(tc, qT, keys, vals, outk, outv, S_, N_):
    nc = tc.nc
    PIECE = min(2048, S_ // 2)  # PSUM tile free size (4 banks at 2048)
    NSUB = PIECE // 512  # matmuls per PSUM tile
    MT = N_ // 128  # number of 128-query tiles
    ST = S_ // 128  # number of 128-chunk tiles (phase 1)
    MM_DT = F32R if MM_DTYPE == "f32r" else F32
    # ckT is split into PIECE-wide sub-tiles so phase-2 matmuls only depend on
    # the phase-1 slice they actually read (earlier pipeline start)
    CKSPLIT = PIECE
    ST_PER_CK = CKSPLIT // 128

    with ExitStack() as ctx:
        const_pool = ctx.enter_context(tc.tile_pool(name="const", bufs=1))
        ident = const_pool.tile([128, 128], F32)
        make_identity(nc, ident[:])
        # chunk-mean keys, transposed, in matmul dtype
        ckTs = [
            const_pool.tile([Dh, CKSPLIT], MM_DT, tag=f"ckT{i}", name=f"ckT{i}")
            for i in range(S_ // CKSPLIT)
        ]

        # ---------------- phase 1: ckT = mean_c(keys).T ----------------
        with (
            tc.tile_pool(name="p1k", bufs=3) as kp,
            tc.tile_pool(name="p1c", bufs=3) as cp,
            tc.tile_pool(name="p1ps", bufs=2, space="PSUM") as pp,
        ):
            for st in range(ST):
                kt = kp.tile([128, CHUNK], F32)
                nc.sync.dma_start(kt[:], keys[st * 128 : (st + 1) * 128, :])
                # chunk-sum over c via pairwise adds on GpSimd (ranking is
                # scale-invariant, so sum stands in for the reference mean)
                t1 = cp.tile([128, CHUNK // 2], F32, tag="t1")
                nc.gpsimd.tensor_add(t1[:], kt[:, : CHUNK // 2], kt[:, CHUNK // 2 :])
                t2 = cp.tile([128, CHUNK // 4], F32, tag="t2")
                nc.gpsimd.tensor_add(t2[:], t1[:, : CHUNK // 4], t1[:, CHUNK // 4 :])
                ck = cp.tile([128, Dh], F32, tag="ck")
                nc.gpsimd.tensor_add(ck[:], t2[:, :Dh], t2[:, Dh:])
                ps = pp.tile([Dh, 128], F32)
                nc.tensor.transpose(ps[:], ck[:], ident[:])
                dst = ckTs[st // ST_PER_CK]
                o = (st % ST_PER_CK) * 128
                nc.scalar.activation(
                    dst[:, o : o + 128], ps[:], mybir.ActivationFunctionType.Copy
                )

        # ---------------- phase 2: scores, top-k, gather ----------------
        # top-k is computed per PCS-wide piece, then merged; smaller pieces
        # start the DVE earlier and pipeline finer than half-row pieces.
        # All NP piece buffers stay live until their find_index8, +1 for
        # cross-M-tile pipelining.
        NP = 4 if S_ >= 4096 else 2
        PCS = S_ // NP  # piece width for the top-k scan
        with (
            tc.tile_pool(name="qm", bufs=2) as qp,
            tc.tile_pool(name="sc", bufs=NP + 1) as sp,
            tc.tile_pool(name="sm", bufs=2) as smp,
            tc.tile_pool(name="g", bufs=1) as gp,
            tc.tile_pool(name="ps2", bufs=2, space="PSUM") as pp2,
        ):
            for m in range(MT):
                qTm = qp.tile([Dh, 128], MM_DT)
                nc.sync.dma_start(qTm[:], qT[:, m * 128 : (m + 1) * 128])

                vv = smp.tile([128, 8 * NP], F32, tag="vv")
                pieces = []
                for pi in range(NP):
                    sc = sp.tile([128, PCS], F32, tag="sc")
                    for pc in range(PCS // PIECE):
                        ps = pp2.tile([128, PIECE], F32)
                        for j in range(NSUB):
                            s0 = pi * PCS + pc * PIECE + j * 512
                            ck_sub = ckTs[s0 // CKSPLIT]
                            o = s0 % CKSPLIT
                            nc.tensor.matmul(
                                ps[:, j * 512 : (j + 1) * 512],
                                lhsT=qTm[:],
                                rhs=ck_sub[:, o : o + 512],
                                start=True,
                                stop=True,
                            )
                        nc.scalar.activation(
                            sc[:, pc * PIECE : (pc + 1) * PIECE],
                            ps[:],
                            mybir.ActivationFunctionType.Copy,
                        )
                    # top-8 values of this piece
                    nc.vector.max(out=vv[:, pi * 8 : (pi + 1) * 8], in_=sc[:])
                    pieces.append(sc)

                # exact top-8 values of the full row
                vtop = smp.tile([128, 8], F32, tag="vtop")
                nc.vector.max(out=vtop[:], in_=vv[:])
                # per-piece positions (0xFFFF when not found / already matched);
                # merging with `min` keeps the lowest global index, which also
                # resolves cross-piece duplicates in jax's tie order
                fs = []
                for pi, sc in enumerate(pieces):
                    ix = smp.tile([128, 8], U16, tag=f"ix{pi}", name=f"ix{pi}")
                    nc.vector.max_index(ix[:], vtop[:], sc[:])
                    f = smp.tile([128, 8], F32, tag=f"f{pi}", name=f"f{pi}")
                    nc.vector.tensor_copy(f[:], ix[:])
                    # global index = ix + pi*PCS (fp32, exact for ints; a miss
                    # is 65535 and stays >= S_, losing every min below)
                    if pi:
                        nc.vector.tensor_scalar_add(f[:], f[:], float(pi * PCS))
                    fs.append(f)
                acc = fs[0]
                for f in fs[1:-1]:
                    nc.vector.tensor_tensor(
                        acc[:], acc[:], f[:], op=mybir.AluOpType.min
                    )
                # final: min(min(acc, S-1), last) — clamp keeps the
                # (astronomically rare) all-pieces-miss case in bounds
                nc.vector.scalar_tensor_tensor(
                    acc[:],
                    acc[:],
                    float(S_ - 1),
                    fs[-1][:],
                    op0=mybir.AluOpType.min,
                    op1=mybir.AluOpType.min,
                )
                comb = smp.tile([128, 8], U32, tag="comb")
                nc.vector.tensor_copy(comb[:], acc[:])

                # gather top-4 chunks of keys and vals by index (2KB each);
                # one indirect DMA per (tensor, j): offsets [128, 1].
                # NOTE: batching offsets as [128, KPC] passes CoreSim but
                # scrambles chunk placement on HW — keep per-column gathers.
                gk = gp.tile([128, KPC * CHUNK], F32, tag="gk")
                gv = gp.tile([128, KPC * CHUNK], F32, tag="gv")
                for j in range(KPC):
                    nc.gpsimd.indirect_dma_start(
                        out=gk[:, j * CHUNK : (j + 1) * CHUNK],
                        out_offset=None,
                        in_=keys[:, :],
                        in_offset=IndirectOffsetOnAxis(ap=comb[:, j : j + 1], axis=0),
                    )
                    nc.gpsimd.indirect_dma_start(
                        out=gv[:, j * CHUNK : (j + 1) * CHUNK],
                        out_offset=None,
                        in_=vals[:, :],
                        in_offset=IndirectOffsetOnAxis(ap=comb[:, j : j + 1], axis=0),
                    )
                nc.sync.dma_start(outk[m * 128 : (m + 1) * 128, :], gk[:])
                nc.sync.dma_start(outv[m * 128 : (m + 1) * 128, :], gv[:])


def build_nc(S_=S, N_=N, debug=False, repeat=1):
    nc = bacc.Bacc("TRN2", target_bir_lowering=False, debug=debug)
    qt_dt = F32R if MM_DTYPE == "f32r" else F32
    qT = nc.dram_tensor("qT", [Dh, N_], qt_dt, kind="ExternalInput").ap()
    keys = nc.dram_tensor("keys", [S_, CHUNK], F32, kind="ExternalInput").ap()
    vals = nc.dram_tensor("vals", [S_, CHUNK], F32, kind="ExternalInput").ap()
    outk = nc.dram_tensor("outk", [N_, KPC * CHUNK], F32, kind="ExternalOutput").ap()
    outv = nc.dram_tensor("outv", [N_, KPC * CHUNK], F32, kind="ExternalOutput").ap()
    with tile.TileContext(nc) as tc:
        retrieval_kernel(tc, qT, keys, vals, outk, outv, S_, N_, repeat=repeat)
    nc.compile()
    return nc


_NC = None
LAST_RESULTS = None  # BassKernelResults of the most recent kernel() call


def kernel(queries, mem_keys, mem_vals):
    global _NC, LAST_RESULTS
    if _NC is None:
        _NC = build_nc()

    q = np.asarray(queries, dtype=np.float32).reshape(N, H, Dh)
    in_maps = []
    for h in range(H):
        in_maps.append(
            {
                "qT": np.ascontiguousarray(q[:, h, :].T),
                "keys": np.ascontiguousarray(
                    np.asarray(mem_keys[h], dtype=np.float32).reshape(S, CHUNK)
                ),
                "vals": np.ascontiguousarray(
                    np.asarray(mem_vals[h], dtype=np.float32).reshape(S, CHUNK)
                ),
            }
        )

    res = run_bass_kernel_spmd(nc=_NC, in_maps=in_maps, core_ids=list(range(H)))
    LAST_RESULTS = res

    ks = np.stack(
        [res.results[h]["outk"].reshape(N, K, Dh) for h in range(H)], axis=1
    ).reshape(N * H, K, Dh)
    vs = np.stack(
        [res.results[h]["outv"].reshape(N, K, Dh) for h in range(H)], axis=1
    ).reshape(N * H, K, Dh)
    return np.stack([ks, vs]).astype(np.float32)




# revision 13
# speedup vs baseline: 18.6011x; 18.6011x over previous
"""Trainium2 Bass kernel for nn_EnhancedExternalMemoryBank (retrieval_knn).

Sharding: one head per NeuronCore (8 heads, 8 cores). Queries are sliced
per-head on the host (layout only); all arithmetic (chunk-sum, score GEMM,
top-k, chunk gather) runs on-device. Host concatenates per-head outputs.

Per-core pipeline (v3 — interleaved head, DVE-floor bound):
  The engine SEQ streams execute in program order, so phase 1 (chunk-sum +
  transpose of mem_keys into the resident ckT [64, S]) is EMITTED
  INTERLEAVED with tile 0's phase-2 work, one 1024-column ckT piece at a
  time. No engine queue is ever head-blocked behind the whole of phase 1:
  matmuls/drains/scans start as soon as their ckT piece exists.
    phase 1 per 128-chunk tile: keys DMA (alternating SP/ACT hwdge
    queues); chunk-sum over c by pairwise adds (level 1 on DVE, which is
    otherwise idle during the head; levels 2-3 on GpSimd; ranking is
    scale-invariant so sum == mean); PE transpose; ACT drain into ckT.
  phase 2 per 128-query tile:
    PE: scores = qT.T @ ckT (exact fp32) into PSUM; ACT drains PSUM->SBUF
    DVE: max8 per S/4-wide piece -> merge to exact top-8 row values;
         find_index8 per piece; strided-min reduce of (index + piece
         offset) resolves cross-piece duplicates in jax's tie order
         (not-found = 65535 loses)
    GPSIMD: indirect DMA gathers the top-4 chunks (2KB each) of keys/vals
            straight from DRAM by index, one chunk column per instruction
    contiguous DMA writes the gathered rows to the DRAM outputs

The kernel is DVE-bound: the two exact full scans (max8 + find_index8) over
the 4096x16384 fp32 score matrix are the floor (~1.09 ms/core); GEMM,
drains, sums, and all DMA overlap underneath it.
"""

import sys

sys.path.insert(0, "/opt/trn_rl_repo")

from contextlib import ExitStack

import numpy as np

import concourse.bass as bass
import concourse.tile as tile
from concourse import bacc, mybir
from concourse.bass import IndirectOffsetOnAxis
from concourse.bass_utils import run_bass_kernel_spmd
from concourse.masks import make_identity

F32 = mybir.dt.float32
U16 = mybir.dt.uint16
U32 = mybir.dt.uint32

# Problem geometry (hardcoded per spec)
H, S, C, Dh = 8, 16384, 8, 64
L, B, Dm = 2048, 2, 512
N = L * B  # 4096 queries
KPC = 4  # chunks retrieved per query (k_per_chunk)
K = KPC * C  # 32
CHUNK = C * Dh  # 512 f32 = 2KB per chunk


def retrieval_kernel(tc, qT, keys, vals, outk, outv, S_, N_, repeat=1):
    for _rep in range(repeat):
        _emit_once(tc, qT, keys, vals, outk, outv, S_, N_)


def _emit_once(tc, qT, keys, vals, outk, outv, S_, N_):
    nc = tc.nc
    PIECE = 1024  # PSUM tile free size (2 banks)
    NSUB = PIECE // 512  # matmuls per PSUM tile
    MT = N_ // 128  # number of 128-query tiles
    ST = S_ // 128  # number of 128-chunk tiles (phase 1)
    CKSPLIT = PIECE  # ckT tile width
    ST_PER_CK = CKSPLIT // 128  # phase-1 tiles per ckT piece
    NCK = S_ // CKSPLIT  # number of ckT pieces
    NP = 4 if S_ >= 4096 else 2  # top-k scan pieces per row
    PCS = S_ // NP  # scan piece width
    CK_PER_P = PCS // CKSPLIT  # ckT pieces per scan piece

    with ExitStack() as ctx:
        const_pool = ctx.enter_context(tc.tile_pool(name="const", bufs=1))
        ident = const_pool.tile([128, 128], F32)
        make_identity(nc, ident[:])
        ckTs = [
            const_pool.tile([Dh, CKSPLIT], F32, tag=f"ckT{i}", name=f"ckT{i}")
            for i in range(NCK)
        ]
        # column p*8+r of the merge buffer gets offset p*PCS added
        poff = const_pool.tile([128, 8 * NP], F32, tag="poff", name="poff")
        for pi in range(NP):
            nc.vector.memset(poff[:, pi * 8 : (pi + 1) * 8], float(pi * PCS))

        hp = ctx.enter_context(tc.tile_pool(name="p1h", bufs=8))
        cp_ = ctx.enter_context(tc.tile_pool(name="p1c", bufs=6))
        pp = ctx.enter_context(tc.tile_pool(name="p1ps", bufs=2, space="PSUM"))
        qp = ctx.enter_context(tc.tile_pool(name="qm", bufs=2))
        sp = ctx.enter_context(tc.tile_pool(name="sc", bufs=NP + 1))
        smp = ctx.enter_context(tc.tile_pool(name="sm", bufs=2))
        gp = ctx.enter_context(tc.tile_pool(name="g", bufs=2))
        pp2 = ctx.enter_context(tc.tile_pool(name="ps2", bufs=2, space="PSUM"))

        def phase1_st(st):
            kt = hp.tile([128, CHUNK], F32)
            eng = nc.sync if st % 2 == 0 else nc.scalar
            eng.dma_start(kt[:], keys[st * 128 : (st + 1) * 128, :])
            # chunk-sum over c by pairwise adds: level 1 on GpSimd (so the
            # kt buffer frees fast), levels 2-3 on DVE (fills its head idle
            # without touching the keys-DMA pipeline)
            t1 = cp_.tile([128, CHUNK // 2], F32, tag="t1")
            if st % 2 == 1:
                nc.vector.tensor_tensor(
                    t1[:],
                    kt[:, : CHUNK // 2],
                    kt[:, CHUNK // 2 :],
                    op=mybir.AluOpType.add,
                )
            else:
                nc.gpsimd.tensor_add(t1[:], kt[:, : CHUNK // 2], kt[:, CHUNK // 2 :])
            t2 = cp_.tile([128, CHUNK // 4], F32, tag="t2")
            if st % 2 == 0:
                nc.gpsimd.tensor_add(t2[:], t1[:, : CHUNK // 4], t1[:, CHUNK // 4 :])
            else:
                nc.vector.tensor_tensor(
                    t2[:],
                    t1[:, : CHUNK // 4],
                    t1[:, CHUNK // 4 :],
                    op=mybir.AluOpType.add,
                )
            ck = cp_.tile([128, Dh], F32, tag="ck")
            nc.vector.tensor_tensor(
                ck[:], t2[:, :Dh], t2[:, Dh:], op=mybir.AluOpType.add
            )
            ps = pp.tile([Dh, 128], F32)
            nc.tensor.transpose(ps[:], ck[:], ident[:])
            dst = ckTs[st // ST_PER_CK]
            o = (st % ST_PER_CK) * 128
            nc.scalar.activation(
                dst[:, o : o + 128], ps[:], mybir.ActivationFunctionType.Copy
            )

        def mm_piece(qTm, sc_piece, cki):
            """Matmuls + drain for ckT piece cki into the right sc slice."""
            ps = pp2.tile([128, PIECE], F32)
            for j in range(NSUB):
                nc.tensor.matmul(
                    ps[:, j * 512 : (j + 1) * 512],
                    lhsT=qTm[:],
                    rhs=ckTs[cki][:, j * 512 : (j + 1) * 512],
                    start=True,
                    stop=True,
                )
            o = (cki % CK_PER_P) * PIECE
            nc.scalar.activation(
                sc_piece[:, o : o + PIECE], ps[:], mybir.ActivationFunctionType.Copy
            )

        def finish_tile(m, pieces, vv):
            """vtop, find_index, merge, gather, output DMA for one tile."""
            vtop = smp.tile([128, 8], F32, tag="vtop")
            nc.vector.max(out=vtop[:], in_=vv[:])
            ixs = smp.tile([128, 8 * NP], U16, tag="ixs")
            for pi, sc_piece in enumerate(pieces):
                nc.vector.max_index(ixs[:, pi * 8 : (pi + 1) * 8], vtop[:], sc_piece[:])
            fx = smp.tile([128, 8 * NP], F32, tag="fx")
            nc.vector.tensor_copy(fx[:], ixs[:])
            # global index = local + piece offset (fp32 exact for ints; a
            # miss is 65535 and stays >= S_, losing every min below)
            nc.vector.tensor_tensor(fx[:], fx[:], poff[:], op=mybir.AluOpType.add)
            acc = smp.tile([128, 8], F32, tag="acc")
            nc.vector.tensor_reduce(
                acc[:],
                fx[:].rearrange("p (a b) -> p b a", a=NP),
                axis=mybir.AxisListType.X,
                op=mybir.AluOpType.min,
            )
            # clamp keeps the (astronomically rare) all-miss case in bounds
            comb = smp.tile([128, 8], U32, tag="comb")
            nc.vector.tensor_scalar_min(comb[:], acc[:], float(S_ - 1))

            # gather top-4 chunks of keys and vals by index (2KB each);
            # one indirect DMA per (tensor, j): offsets [128, 1].
            # NOTE: batching offsets as [128, KPC] passes CoreSim but
            # scrambles chunk placement on HW — keep per-column gathers.
            gk = gp.tile([128, KPC * CHUNK], F32, tag="gk")
            gv = gp.tile([128, KPC * CHUNK], F32, tag="gv")
            r0 = m * 128
            for j in range(KPC):
                nc.gpsimd.indirect_dma_start(
                    out=gk[:, j * CHUNK : (j + 1) * CHUNK],
                    out_offset=None,
                    in_=keys[:, :],
                    in_offset=IndirectOffsetOnAxis(ap=comb[:, j : j + 1], axis=0),
                )
                nc.gpsimd.indirect_dma_start(
                    out=gv[:, j * CHUNK : (j + 1) * CHUNK],
                    out_offset=None,
                    in_=vals[:, :],
                    in_offset=IndirectOffsetOnAxis(ap=comb[:, j : j + 1], axis=0),
                )
                # stream each gathered chunk column out as soon as it lands
                nc.sync.dma_start(
                    outk[r0 : r0 + 128, j * CHUNK : (j + 1) * CHUNK],
                    gk[:, j * CHUNK : (j + 1) * CHUNK],
                )
                nc.sync.dma_start(
                    outv[r0 : r0 + 128, j * CHUNK : (j + 1) * CHUNK],
                    gv[:, j * CHUNK : (j + 1) * CHUNK],
                )

        # ---- head: phase 1 interleaved with tile 0's scoring/scans ----
        qTm = qp.tile([Dh, 128], F32)
        nc.scalar.dma_start(qTm[:], qT[:, 0:128])
        vv = smp.tile([128, 8 * NP], F32, tag="vv")
        pieces = []
        for cki in range(NCK):
            for st in range(cki * ST_PER_CK, (cki + 1) * ST_PER_CK):
                phase1_st(st)
            if cki % CK_PER_P == 0:
                pieces.append(sp.tile([128, PCS], F32, tag="sc", name="scp"))
            mm_piece(qTm, pieces[-1], cki)
            if cki % CK_PER_P == CK_PER_P - 1:
                pi = cki // CK_PER_P
                nc.vector.max(out=vv[:, pi * 8 : (pi + 1) * 8], in_=pieces[pi][:])
        finish_tile(0, pieces, vv)

        # ---------------- steady state: tiles 1..MT-1 ----------------
        for m in range(1, MT):
            qTm = qp.tile([Dh, 128], F32)
            nc.scalar.dma_start(qTm[:], qT[:, m * 128 : (m + 1) * 128])
            vv = smp.tile([128, 8 * NP], F32, tag="vv")
            pieces = []
            for cki in range(NCK):
                if cki % CK_PER_P == 0:
                    pieces.append(sp.tile([128, PCS], F32, tag="sc", name="scp"))
                mm_piece(qTm, pieces[-1], cki)
                if cki % CK_PER_P == CK_PER_P - 1:
                    pi = cki // CK_PER_P
                    nc.vector.max(out=vv[:, pi * 8 : (pi + 1) * 8], in_=pieces[pi][:])
            finish_tile(m, pieces, vv)


def build_nc(S_=S, N_=N, debug=False, repeat=1):
    nc = bacc.Bacc("TRN2", target_bir_lowering=False, debug=debug)
    qT = nc.dram_tensor("qT", [Dh, N_], F32, kind="ExternalInput").ap()
    keys = nc.dram_tensor("keys", [S_, CHUNK], F32, kind="ExternalInput").ap()
    vals = nc.dram_tensor("vals", [S_, CHUNK], F32, kind="ExternalInput").ap()
    outk = nc.dram_tensor("outk", [N_, KPC * CHUNK], F32, kind="ExternalOutput").ap()
    outv = nc.dram_tensor("outv", [N_, KPC * CHUNK], F32, kind="ExternalOutput").ap()
    with tile.TileContext(nc) as tc:
        retrieval_kernel(tc, qT, keys, vals, outk, outv, S_, N_, repeat=repeat)
    nc.compile()
    return nc


_NC = None
LAST_RESULTS = None  # BassKernelResults of the most recent kernel() call


def kernel(queries, mem_keys, mem_vals):
    global _NC, LAST_RESULTS
    if _NC is None:
        _NC = build_nc()

    q = np.asarray(queries, dtype=np.float32).reshape(N, H, Dh)
    in_maps = []
    for h in range(H):
        in_maps.append(
            {
                "qT": np.ascontiguousarray(q[:, h, :].T),
                "keys": np.ascontiguousarray(
                    np.asarray(mem_keys[h], dtype=np.float32).reshape(S, CHUNK)
                ),
                "vals": np.ascontiguousarray(
                    np.asarray(mem_vals[h], dtype=np.float32).reshape(S, CHUNK)
                ),
            }
        )

    res = run_bass_kernel_spmd(nc=_NC, in_maps=in_maps, core_ids=list(range(H)))
    LAST_RESULTS = res

    ks = np.stack(
        [res.results[h]["outk"].reshape(N, K, Dh) for h in range(H)], axis=1
    ).reshape(N * H, K, Dh)
    vs = np.stack(
        [res.results[h]["outv"].reshape(N, K, Dh) for h in range(H)], axis=1
    ).reshape(N * H, K, Dh)
    return np.stack([ks, vs]).astype(np.float32)
